# revision 26
# baseline (speedup 1.0000x reference)
"""Trainium2 Bass kernel for nn_LRSVConv (low-rank spatially-varying conv).

Computes, for full inputs
    x            [8, 32, 256, 256]  f32
    conv_w       [192, 32, 3, 3]    f32   (192 = RANK(3) * C_OUT(64))
    kernel_weight[2, 256, 256]      f32
the reference:
    y   = conv2d(x, conv_w, stride 1, pad 1)      # [8, 192, 256, 256]
    y   = y.reshape(8, 3, 64, 256, 256)
    out = y[:,0] + kw[0]*y[:,1] + kw[1]*y[:,2]    # [8, 64, 256, 256]

Strategy: spatial (H) sharding across 8 cores - each core computes a band of
32 output rows for ALL batches, so the per-pixel blend weights (which are
batch-independent) are loaded once per core and reused 8x.

Design (measured ~163 us HW exec vs 364 us for the full-array f32r
baseline; rel err 4.2e-3 vs the 2e-2 gate):
  - bf16 inputs/weights (host-converted; f32 PSUM accumulation).
  - PE column tiling (128x64 mode): every matmul has M=64 and targets one
    PSUM partition half; the two column tiles stream CONCURRENTLY, so a
    [96,64,512] pair completes in the time of one [96,128,512]. The PE does
    ONLY the 18 conv matmuls per supertile (9 concurrent pair-slots,
    ~2.2us) - the floor for this decomposition: 2 M-tiles x 3 K-steps
    cannot shrink (K=288 > 2x128), and fp8 DoubleRow breaks the accuracy
    gate (measured 3.7e-2).
  - Per supertile t (4 image rows = 2 blocks q of 512 px): accumulate
    rank r1 -> AB[:, 0:512], r2 -> AB[:, 512:1024], r0 -> C; PSUM ends up
    rank-aligned on partition (64q+c) with no cross-partition traffic, so
    no identity-matmul folds and no transpose are needed anywhere.
  - blend fold spread across the otherwise-idle engines:
      DVE:    m = AB * svAB      [128,1024] psum*sbuf->sbuf  (~1.15us)
      DVE:    s = C + m1         [128,512]  psum+sbuf->sbuf  (~0.69us)
      GPSIMD: out_sb = s + m2    [128,512]  sbuf, bf16 out   (~1.26us)
  - software-pipelined: supertile t-1's fold ops are emitted after
    supertile t's conv matmuls, so the PE never waits on the DVE multiply;
    tmp bufs=4 ride out transient GPSIMD lag.
  - DMA: input band for batch b+1 prefetched before batch b's output DMAs
    enter the queues; startup-critical first band split 3-way for queue
    parallelism. dma_start count is kept low on purpose - each costs
    ~0.6us of serial descriptor-gen on the Sync engine (finer splitting
    measurably regresses: Sync saturates and the PE drops out of its
    fast p-state).
"""

import os

import numpy as np
from ml_dtypes import bfloat16 as np_bf16

B, C_IN, C_OUT, RANK, IMG = 8, 32, 64, 3, 256
N_CORES = 8
BAND = IMG // N_CORES          # 32 output rows per core
WP = IMG + 2                   # padded width 258
ROWS_IN = BAND + 2             # input rows needed per band (with halo)
SUPER = 8                      # supertiles per (batch, band): 4 rows each
SROWS = BAND // SUPER          # 4 image rows per supertile
NBLK = 512                     # pixels per matmul block (2 image rows)

_F32 = np.float32

NB = int(os.environ.get("KERNEL_NB", str(B)))  # batches to process (debug knob)


def _build_bass():
    import concourse.mybir as mybir
    import concourse.tile as tile
    from concourse import bacc

    f32 = mybir.dt.float32
    bf16 = mybir.dt.bfloat16
    nc = bacc.Bacc("TRN2", target_bir_lowering=False, debug=False)

    xs_t = nc.dram_tensor("xs", (B, C_IN, ROWS_IN * WP), bf16, kind="ExternalInput")
    # wc[(kh,cin), (r,kw), c]: 9 column-tile stationaries of 64 channels
    wc_t = nc.dram_tensor("wc", (96, 9, 64), bf16, kind="ExternalInput")
    # svAB[(q,c), t, (s,j)]: per-pixel blend weights for ranks 1 (s=0), 2 (s=1)
    svb_t = nc.dram_tensor("svb", (128, SUPER, 2 * NBLK), bf16, kind="ExternalInput")
    out_t = nc.dram_tensor("out", (B, C_OUT, BAND, IMG), bf16, kind="ExternalOutput")

    xs = xs_t.ap()
    out_r = out_t.ap().rearrange(
        "b c (t q r) w -> b q c t (r w)", t=SUPER, q=2, r=SROWS // 2
    )

    with tile.TileContext(nc) as tc:
        with (
            tc.tile_pool(name="const", bufs=1) as cpool,
            tc.tile_pool(name="imcol", bufs=2) as ipool,
            tc.tile_pool(name="psab", bufs=3, space="PSUM") as abpool,
            tc.tile_pool(name="psc", bufs=2, space="PSUM") as cpool_ps,
            tc.tile_pool(name="tmp", bufs=4) as tpool,
            tc.tile_pool(name="outp", bufs=4) as opool,
        ):
            wc_sb = cpool.tile([96, 9, 64], bf16)
            nc.sync.dma_start(wc_sb[:], wc_t.ap())

            def load_imcol(b, nchunk=1):
                # nchunk>1 only for the startup load (more queue parallelism);
                # steady-state prefetches stay coarse to spare the Sync engine
                ch = BAND * WP // nchunk
                t = ipool.tile([96, BAND * WP], bf16, tag="imcol")
                for kh in range(3):
                    for j in range(nchunk):
                        nc.sync.dma_start(
                            t[32 * kh : 32 * kh + 32, ch * j : ch * (j + 1)],
                            xs[b, :, kh * WP + ch * j : kh * WP + ch * (j + 1)],
                        )
                return t

            imcol = load_imcol(0, nchunk=3)

            # per-supertile sv tiles, loaded after the first input band so
            # they never gate the first matmuls; the first two are split so
            # they arrive before the supertile-0/1 blend multiplies
            svb_sbs = []
            for t in range(SUPER):
                sv = cpool.tile([128, 2 * NBLK], bf16, tag=f"svb{t}")
                nsp = 2 if t < 2 else 1
                for j in range(nsp):
                    w = 2 * NBLK // nsp
                    nc.sync.dma_start(
                        sv[:, w * j : w * (j + 1)],
                        svb_t.ap()[:, t, w * j : w * (j + 1)],
                    )
                svb_sbs.append(sv)

            def emit_conv(imv, t):
                """18 column-tiled conv matmuls for supertile t; returns (AB, C)."""
                ab = abpool.tile([128, 2 * NBLK], f32, tag="ab")
                c = cpool_ps.tile([128, NBLK], f32, tag="c")
                hl = SROWS * t
                # AB matmuls first, C matmuls last: the C bank (bufs=2) has
                # a WAR on the DVE s-add two supertiles back, so its first
                # write is deferred ~1.3us into the supertile
                for kw in range(3):
                    st, sp = kw == 0, kw == 2
                    for q in range(2):
                        rhs = imv[:, hl + 2 * q : hl + 2 * q + 2, kw : kw + IMG]
                        o = 64 * q
                        nc.tensor.matmul(
                            ab[o : o + 64, 0:NBLK],
                            wc_sb[:, 3 * 1 + kw, :], rhs, start=st, stop=sp,
                        )
                        nc.tensor.matmul(
                            ab[o : o + 64, NBLK : 2 * NBLK],
                            wc_sb[:, 3 * 2 + kw, :], rhs, start=st, stop=sp,
                        )
                for kw in range(3):
                    st, sp = kw == 0, kw == 2
                    for q in range(2):
                        rhs = imv[:, hl + 2 * q : hl + 2 * q + 2, kw : kw + IMG]
                        o = 64 * q
                        nc.tensor.matmul(
                            c[o : o + 64, :],
                            wc_sb[:, 3 * 0 + kw, :], rhs, start=st, stop=sp,
                        )
                return ab, c

            def emit_blend_mult(ab, t):
                """DVE: m = AB * svAB  (psum f32 x sbuf f32 -> sbuf f32)."""
                m = tpool.tile([128, 2 * NBLK], f32, tag="m")
                nc.vector.tensor_tensor(
                    m[:], ab, svb_sbs[t][:], mybir.AluOpType.mult
                )
                return m

            def emit_fold_out(c, m, b, t):
                """DVE: s = C + m1; GPSIMD: out = s + m2; DMA out."""
                s = tpool.tile([128, NBLK], f32, tag="s")
                nc.vector.tensor_tensor(s[:], c, m[:, 0:NBLK], mybir.AluOpType.add)
                out_sb = opool.tile([128, NBLK], bf16, tag="out_sb")
                nc.gpsimd.tensor_tensor(
                    out_sb[:], s[:], m[:, NBLK : 2 * NBLK], mybir.AluOpType.add
                )
                for q in range(2):
                    nc.sync.dma_start(
                        out_r[b, q, :, t, :], out_sb[64 * q : 64 * q + 64, :]
                    )

            pend = None  # (C, m, b, t) of the previous supertile
            for b in range(NB):
                imcol_nxt = load_imcol(b + 1) if b + 1 < NB else None
                imv = imcol.rearrange("p (h w) -> p h w", w=WP)
                for t in range(SUPER):
                    ab, c = emit_conv(imv, t)
                    m = emit_blend_mult(ab, t)
                    if pend is not None:
                        emit_fold_out(*pend)
                    pend = (c, m, b, t)
                imcol = imcol_nxt
            emit_fold_out(*pend)
    nc.compile()
    return nc


_CACHE = {}


def _get_bass():
    if "nc" not in _CACHE:
        _CACHE["nc"] = _build_bass()
    return _CACHE["nc"]


def _prep_shards(x, conv_w, kernel_weight):
    x = np.asarray(x, dtype=_F32)
    conv_w = np.asarray(conv_w, dtype=_F32)
    kernel_weight = np.asarray(kernel_weight, dtype=_F32)

    x_pad = np.pad(x, ((0, 0), (0, 0), (1, 1), (1, 1))).astype(np_bf16)
    # wc[(kh,cin), (r,kw), c] from conv_w[(r c), cin, kh, kw]
    wc = np.ascontiguousarray(
        conv_w.reshape(RANK, C_OUT, C_IN, 3, 3)
        .transpose(3, 2, 0, 4, 1)
        .reshape(96, 9, 64)
    ).astype(np_bf16)

    in_maps = []
    for i in range(N_CORES):
        h0 = BAND * i
        shard = np.ascontiguousarray(
            x_pad[:, :, h0 : h0 + ROWS_IN, :]
        ).reshape(B, C_IN, ROWS_IN * WP)
        band = kernel_weight[:, h0 : h0 + BAND, :]          # [2, 32, 256]
        # svAB[64q+c, t, (s,j)] = band[s, 4t+2q+(j//256), j%256]
        tmp = band.reshape(2, SUPER, 2, NBLK)               # [s, t, q, j]
        svb = np.broadcast_to(
            tmp.transpose(2, 1, 0, 3)[:, None],             # [q, 1, t, s, j]
            (2, C_OUT, SUPER, 2, NBLK),
        ).reshape(128, SUPER, 2 * NBLK)
        svb = np.ascontiguousarray(svb).astype(np_bf16)
        in_maps.append({"xs": shard, "wc": wc, "svb": svb})
    return in_maps


def run(inputs, trace=False):
    """Run the sharded bass kernel; returns (out_full, BassKernelResults)."""
    from concourse.bass_utils import run_bass_kernel_spmd

    in_maps = _prep_shards(**inputs)
    nc = _get_bass()
    res = run_bass_kernel_spmd(
        nc, in_maps, core_ids=list(range(N_CORES)), trace=trace
    )
    out = np.empty((B, C_OUT, IMG, IMG), dtype=_F32)
    for i in range(N_CORES):
        out[:, :, BAND * i : BAND * (i + 1), :] = res.results[i]["out"]
    return out, res


def kernel(x, conv_w, kernel_weight):
    out, _ = run({"x": x, "conv_w": conv_w, "kernel_weight": kernel_weight})
    return out
